# revision 16
# baseline (speedup 1.0000x reference)
"""AdaPT_Linear (per-tensor int8-quantized linear) on 8 trn2 NeuronCores.

Strategy (data-parallel over rows of x):
  - Host passes x.T shards [1024, 2048] and w.T [1024, 1024] (replicated),
    so SBUF loads land with the contraction (IN) axis on partitions and no
    on-device transposes are needed.
  - Quantized int8 values are exact in bf16; int8xint8 products accumulate
    exactly in fp32 PSUM (max |acc| = 127*127*1024 < 2^24), so a bf16
    matmul reproduces the reference int32 matmul bit-exactly.
  - Per-tensor abs-max of x -> AllReduce(max) across the 8 cores; w/bias
    are replicated so their scales are computed locally.
  - round-half-even matches jnp.round via the +/-1.5*2^23 magic constant;
    clip is a no-op because the scale uses the global abs-max.
  - The w/bias loads are gated behind the collective-input DMA so the x
    load and the tiny abs-max bounce get the DMA engines to themselves;
    w work then runs in the AllReduce's shadow.
  - Post-collective, x is quantized in half-tiles feeding a k-outer
    matmul schedule that spreads 8 accumulation groups across the 8 PSUM
    banks, so the PE starts as soon as the first quantized tile exists.
"""
import numpy as np

import concourse.bacc as bacc
import concourse.mybir as mybir
import concourse.tile as tile
from concourse import bass_isa
from concourse import library_config
from concourse.tile_rust import add_dep_helper
from concourse.bass_utils import run_bass_kernel_spmd

N_CORES = 8
N_ROWS = 16384
SIZE_IN = 1024
SIZE_OUT = 1024
ROWS_PER_CORE = N_ROWS // N_CORES          # 2048
K_TILES = SIZE_IN // 128                   # 8
N_CHUNKS = SIZE_OUT // 512                 # 2
R_HALF = ROWS_PER_CORE // 2                # 1024 rows per half
MAGIC = 12582912.0                         # 1.5 * 2**23: round-half-even trick
MAXV = 127.0

F32 = mybir.dt.float32
BF16 = mybir.dt.bfloat16


def build_nc():
    nc = bacc.Bacc(None, target_bir_lowering=False, debug=False,
                   num_devices=N_CORES)

    xt_ext = nc.declare_dram_parameter("xt", [SIZE_IN, ROWS_PER_CORE], F32,
                                       isOutput=False)
    wt_ext = nc.declare_dram_parameter("wt", [SIZE_IN, SIZE_OUT], F32,
                                       isOutput=False)
    b_ext = nc.declare_dram_parameter("bias", [1, SIZE_OUT], F32,
                                      isOutput=False)
    out_ext = nc.declare_dram_parameter("out", [ROWS_PER_CORE, SIZE_OUT], F32,
                                        isOutput=True)

    with tile.TileContext(nc) as tc:
        with (
            tc.tile_pool(name="big", bufs=1) as big,
            tc.tile_pool(name="stats", bufs=1) as stats,
            tc.tile_pool(name="ostage", bufs=8) as ostage,
            tc.tile_pool(name="psum", bufs=8, space="PSUM") as psum_pool,
            tc.tile_pool(name="dram", bufs=1, space="DRAM") as dram,
        ):
            xt_sb = [big.tile([128, ROWS_PER_CORE], F32, tag=f"xt{k}",
                              name=f"xt{k}") for k in range(K_TILES)]
            qxt_sb = [big.tile([128, ROWS_PER_CORE], BF16, tag=f"qxt{k}",
                               name=f"qxt{k}") for k in range(K_TILES)]
            wt_sb = [big.tile([128, SIZE_OUT], F32, tag=f"wt{k}",
                              name=f"wt{k}") for k in range(K_TILES)]
            qwt_sb = [big.tile([128, SIZE_OUT], BF16, tag=f"qwt{k}",
                               name=f"qwt{k}") for k in range(K_TILES)]

            xslots = stats.tile([128, 2 * K_TILES], F32, tag="xslots")
            wslots = stats.tile([128, K_TILES], F32, tag="wslots")
            xmax_l = stats.tile([128, 1], F32, tag="xmax_l")
            xmax_ar = stats.tile([128, 1], F32, tag="xmax_ar")
            xmax_g = stats.tile([128, 1], F32, tag="xmax_g")
            wmax_l = stats.tile([128, 1], F32, tag="wmax_l")
            wmax_g = stats.tile([128, 1], F32, tag="wmax_g")
            rx = stats.tile([128, 1], F32, tag="rx")
            rw = stats.tile([128, 1], F32, tag="rw")
            sa = stats.tile([128, 1], F32, tag="sa")
            sw = stats.tile([128, 1], F32, tag="sw")
            sd_t = stats.tile([128, 1], F32, tag="sd")
            b_sb = stats.tile([1, SIZE_OUT], F32, tag="b_sb")
            bmax = stats.tile([1, 1], F32, tag="bmax")
            rb = stats.tile([1, 1], F32, tag="rb")
            sb_t = stats.tile([1, 1], F32, tag="sb")
            bq = stats.tile([1, SIZE_OUT], F32, tag="bq")
            bval = stats.tile([1, SIZE_OUT], F32, tag="bval")
            bias_full = stats.tile([128, SIZE_OUT], F32, tag="bias_full")
            magic_c = stats.tile([128, 1], F32, tag="magic_c")

            cc_in = dram.tile([128, 1], F32, tag="cc_in")
            cc_out = dram.tile([128 * N_CORES, 1], F32, tag="cc_out")
            ag_sb = stats.tile([128, N_CORES], F32, tag="ag_sb")

            nc.vector.memset(magic_c[:], MAGIC)
            # preload the gpsimd ucode library used by partition_all_reduce /
            # partition_broadcast so its reload overlaps the x load
            nc.gpsimd.load_library(library_config.attn)

            # ---- load x shard (transposed layout) + per-slab |max|.
            #      16 half-tile DMAs alternating across both HWDGE engines
            #      (sync + scalar) so dispatch ramps faster and the abs-max
            #      reduces trail the data closely - the collective trigger
            #      fires as early as possible. ----
            for k in range(K_TILES):
                for h in range(2):
                    sl = slice(h * R_HALF, (h + 1) * R_HALF)
                    eng = nc.sync if (2 * k + h) % 2 == 0 else nc.scalar
                    eng.dma_start(xt_sb[k][:, sl],
                                  xt_ext[k * 128:(k + 1) * 128, sl])
                    nc.vector.tensor_reduce(
                        xslots[:, 2 * k + h:2 * k + h + 1], xt_sb[k][:, sl],
                        axis=mybir.AxisListType.X,
                        op=mybir.AluOpType.max, apply_absolute_value=True)
            nc.vector.tensor_reduce(
                xmax_l[:], xslots[:], axis=mybir.AxisListType.X,
                op=mybir.AluOpType.max)

            # ---- global abs-max of x across the 8 cores.
            #      Pre-mesh: collapse partitions (input becomes uniform) and
            #      pre-divide by 127, so post-mesh sa = reciprocal directly.
            #      AllGather (single-phase mesh, cheaper than AllReduce);
            #      each rank's 512B block is 128 copies of its scalar, so a
            #      [128, 8] strided readback (out[p, r] = dram[r*128 + p])
            #      yields every rank's value on every partition and one free-
            #      axis reduce gives the broadcast global max - no partition
            #      collapse after the mesh. ----
            nc.gpsimd.partition_all_reduce(
                xmax_g[:], xmax_l[:], channels=128,
                reduce_op=bass_isa.ReduceOp.max)
            nc.vector.tensor_scalar_mul(xmax_g[:], xmax_g[:], 1.0 / MAXV)
            cc_in_dma = nc.sync.dma_start(cc_in[:], xmax_g[:])
            nc.gpsimd.collective_compute(
                "AllGather", mybir.AluOpType.bypass,
                replica_groups=[list(range(N_CORES))],
                ins=[cc_in.opt()], outs=[cc_out.opt()])
            cc_view = cc_out.rearrange("(r p) one -> p (r one)", p=128)
            with tc.high_priority():
                nc.sync.dma_start(ag_sb[:], cc_view)
                nc.vector.tensor_reduce(
                    xmax_ar[:], ag_sb[:], axis=mybir.AxisListType.X,
                    op=mybir.AluOpType.max)
                nc.vector.reciprocal(sa[:], xmax_ar[:])

            # ---- weight + bias load, gated behind the collective input DMA
            #      so they run in the AllReduce's shadow ----
            for k in range(K_TILES):
                wd = nc.sync.dma_start(wt_sb[k][:],
                                       wt_ext[k * 128:(k + 1) * 128, :])
                add_dep_helper(wd.ins, cc_in_dma.ins, sync=True,
                               reason="w load after abs-max bounce")
                nc.vector.tensor_reduce(
                    wslots[:, k:k + 1], wt_sb[k][:], axis=mybir.AxisListType.X,
                    op=mybir.AluOpType.max, apply_absolute_value=True)
            bd = nc.sync.dma_start(b_sb[:], b_ext[:])
            add_dep_helper(bd.ins, cc_in_dma.ins, sync=True,
                           reason="bias load after abs-max bounce")
            nc.vector.tensor_reduce(
                wmax_l[:], wslots[:], axis=mybir.AxisListType.X,
                op=mybir.AluOpType.max)
            nc.gpsimd.partition_all_reduce(
                wmax_g[:], wmax_l[:], channels=128,
                reduce_op=bass_isa.ReduceOp.max)
            nc.vector.tensor_scalar_mul(wmax_g[:], wmax_g[:], 1.0 / MAXV)
            nc.vector.reciprocal(sw[:], wmax_g[:])

            # dequant scale: 1/(sa*sw) = (xmax/127)*(wmax/127)
            nc.vector.tensor_tensor(
                sd_t[:], xmax_ar[:], wmax_g[:], op=mybir.AluOpType.mult)

            # ---- quantize w to bf16 in place (local scale, pre-collective) --
            for k in range(K_TILES):
                nc.scalar.activation(
                    wt_sb[k][:], wt_sb[k][:],
                    mybir.ActivationFunctionType.Identity,
                    bias=magic_c[:], scale=sw[:])
                nc.vector.tensor_scalar(
                    qwt_sb[k][:], wt_sb[k][:], -MAGIC, None,
                    op0=mybir.AluOpType.add, op1=mybir.AluOpType.bypass)

            # ---- bias: quantize + dequantize locally, broadcast row ----
            nc.vector.tensor_reduce(
                bmax[:], b_sb[:], axis=mybir.AxisListType.X,
                op=mybir.AluOpType.max, apply_absolute_value=True)
            nc.vector.reciprocal(rb[:], bmax[:])
            nc.vector.tensor_scalar_mul(sb_t[:], rb[:], MAXV)
            nc.vector.tensor_scalar(
                bq[:], b_sb[:], sb_t[:], MAGIC,
                op0=mybir.AluOpType.mult, op1=mybir.AluOpType.add)
            nc.vector.tensor_scalar(
                bq[:], bq[:], -MAGIC, None, op0=mybir.AluOpType.add,
                op1=mybir.AluOpType.bypass)
            # bias value row = qb/sb = qb * bmax / 127
            nc.vector.tensor_scalar(
                bval[:], bq[:], bmax[:], 1.0 / MAXV,
                op0=mybir.AluOpType.mult, op1=mybir.AluOpType.mult)
            nc.gpsimd.partition_broadcast(bias_full[:], bval[:], channels=128)

            # ---- quantize x (k-major, matching PE consumption order; the
            #      first k-tile in quarter-slabs so the PE starts sooner) ----
            def quantize_x(cols):
                for kq in range(K_TILES):
                    slabs = ([slice(cols.start, cols.start + 512),
                              slice(cols.start + 512, cols.stop)]
                             if kq == 0 and cols.start == 0 else [cols])
                    for sl in slabs:
                        nc.scalar.activation(
                            xt_sb[kq][:, sl], xt_sb[kq][:, sl],
                            mybir.ActivationFunctionType.Identity,
                            bias=magic_c[:], scale=sa[:])
                        nc.vector.tensor_scalar(
                            qxt_sb[kq][:, sl], xt_sb[kq][:, sl], -MAGIC, None,
                            op0=mybir.AluOpType.add,
                            op1=mybir.AluOpType.bypass)

            for half in range(2):
                quantize_x(slice(half * R_HALF, (half + 1) * R_HALF))

            # ---- matmul: k-outer over 8 PSUM banks (one per row-tile), so
            #      the PE consumes quantized k-tiles as they are produced ----
            blocks = [(0, 0, range(8)), (0, 1, range(8)),
                      (1, 0, range(8)), (1, 1, range(8))]
            for half, n, rng in blocks:
                    ps = {r: psum_pool.tile([128, 512], F32, tag="ps",
                                            name=f"ps_h{half}n{n}r{r}")
                          for r in rng}
                    for k in range(K_TILES):
                        last = (k == K_TILES - 1)
                        for r in rng:
                            col0 = half * R_HALF + r * 128
                            nc.tensor.matmul(
                                ps[r][:],
                                qxt_sb[k][:, col0:col0 + 128],
                                qwt_sb[k][:, n * 512:(n + 1) * 512],
                                start=(k == 0), stop=last)
                            if last:
                                # dequant+store right after each group's
                                # final matmul so the tail doesn't serialize
                                ot = ostage.tile([128, 512], F32, tag="ot")
                                nc.vector.scalar_tensor_tensor(
                                    ot[:], ps[r][:], sd_t[:],
                                    bias_full[:, n * 512:(n + 1) * 512],
                                    op0=mybir.AluOpType.mult,
                                    op1=mybir.AluOpType.add)
                                row0 = half * R_HALF + r * 128
                                nc.sync.dma_start(
                                    out_ext[row0:row0 + 128,
                                            n * 512:(n + 1) * 512], ot[:])

    nc.finalize()
    return nc


_NC_CACHE = None


def _get_nc():
    global _NC_CACHE
    if _NC_CACHE is None:
        _NC_CACHE = build_nc()
    return _NC_CACHE


def kernel(x, weight, bias):
    assert x.shape == (N_ROWS, SIZE_IN) and x.dtype == np.float32
    nc = _get_nc()
    wt = np.ascontiguousarray(weight.T)
    b2 = np.ascontiguousarray(bias.reshape(1, SIZE_OUT))
    in_maps = []
    for c in range(N_CORES):
        shard = np.ascontiguousarray(
            x[c * ROWS_PER_CORE:(c + 1) * ROWS_PER_CORE, :].T)
        in_maps.append({"xt": shard, "wt": wt, "bias": b2})
    res = run_bass_kernel_spmd(nc, in_maps, core_ids=list(range(N_CORES)))
    return np.concatenate([res.results[c]["out"] for c in range(N_CORES)],
                          axis=0)
